# revision 17
# baseline (speedup 1.0000x reference)
"""MoE expert-parallel kernel for Trainium2 (8 NeuronCores).

Strategy:
  - Host: route tokens to experts (stable sort by dispatch_order). Experts are
    assigned to (core, slot) pairs by descending token count: slot j of core c
    gets the (8*j + c)-th most-loaded expert, so all cores see nearly identical
    work and slot j's capacity cap_j = max over cores of its count (tight).
  - Device (SPMD, 8 cores, 8 expert slots/core):
    per slot: HT = gelu(w1^T-tiled @ XT + b1) computed transposed [F, tokens],
    then Y = HT^T @ w2 + b2 [tokens, D]; bf16 operands, fp32 PSUM accumulation.
  - Host: scatter per-expert outputs back to original token order.

No cross-core collectives: each core owns a disjoint set of experts, hence a
disjoint set of output token rows.
"""

import sys

import numpy as np
import ml_dtypes

for _p in ("/opt/trn_rl_repo",):
    if _p not in sys.path:
        sys.path.insert(0, _p)

_BF16 = ml_dtypes.bfloat16

NUM_EXPERTS = 64
N_CORES = 8
E_LOCAL = NUM_EXPERTS // N_CORES  # 8 expert slots per core
D = 512
F = 2048
KD = D // 128   # 4 contraction tiles for layer 1
KF = F // 128   # 16 contraction tiles for layer 2

_nc_cache = {}


def _slot_geometry(caps):
    """Per-slot column offsets for xt and row offsets for y."""
    xoff = [0]
    yoff = [0]
    for c in caps:
        xoff.append(xoff[-1] + c)
        yoff.append(yoff[-1] + (-(-c // 128)) * 128)
    return xoff, yoff


def _build_nc(caps):
    """Build + compile the SPMD Bass program for per-slot capacities `caps`."""
    import concourse.bacc as bacc
    import concourse.bass as bass
    import concourse.mybir as mybir
    import concourse.tile as tile

    fp32 = mybir.dt.float32
    bf16 = mybir.dt.bfloat16

    xoff, yoff = _slot_geometry(caps)
    XCOLS = xoff[-1]
    YROWS = yoff[-1]
    CAPMAX = max(caps)

    nc = bacc.Bacc("TRN2", target_bir_lowering=False, debug=False)

    # xt/w1r/w2 are partition-major: one contiguous run per partition per
    # transfer -> 128 large DMA descriptors instead of 512-2048 small ones.
    xt_d = nc.dram_tensor("xt", [128, KD * XCOLS], bf16, kind="ExternalInput")
    w1a_d = nc.dram_tensor("w1a", [KD, 128, F], bf16, kind="ExternalInput")
    w1r_d = nc.dram_tensor(
        "w1r", [E_LOCAL - 1, 128, KD * F], bf16, kind="ExternalInput"
    )
    w2_d = nc.dram_tensor("w2", [E_LOCAL, 128, KF * D], bf16, kind="ExternalInput")
    b1_d = nc.dram_tensor("b1", [E_LOCAL, 128, KF], fp32, kind="ExternalInput")
    b2_d = nc.dram_tensor("b2", [E_LOCAL, D], fp32, kind="ExternalInput")
    y_d = nc.dram_tensor("y", [YROWS, D], fp32, kind="ExternalOutput")

    with tile.TileContext(nc) as tc:
        with (
            tc.tile_pool(name="wpool", bufs=2) as wp,
            tc.tile_pool(name="xpool", bufs=2) as xp,
            tc.tile_pool(name="hpool", bufs=2) as hp,
            tc.tile_pool(name="ypool", bufs=4) as yp,
            tc.tile_pool(name="bias", bufs=1) as bp,
            tc.tile_pool(name="psh", bufs=4, space="PSUM") as psh,
            tc.tile_pool(name="psy", bufs=3, space="PSUM") as psy,
        ):
            w1_sbs = [None] * E_LOCAL
            w2_sbs = [None] * E_LOCAL
            xt_sbs = [None] * E_LOCAL

            def load_slot(e, first):
                # Single HWDGE ring (SP): FIFO start order + packet-level
                # round-robin. Critical startup transfers (xt0, w1_0 chunks)
                # are issued first; everything else queues behind them.
                cap = caps[e]
                xt_sb = xp.tile([128, KD * cap], bf16, tag="xt")
                nc.sync.dma_start(
                    out=xt_sb[:],
                    in_=xt_d[:, KD * xoff[e]:KD * xoff[e + 1]],
                )
                w1_sb = wp.tile([128, KD, F], bf16, tag="w1")
                if first:
                    # progressive f-blocks so PE starts after ~1 MB
                    for f0 in range(0, F, 512):
                        nc.sync.dma_start(
                            out=w1_sb[:, :, f0:f0 + 512],
                            in_=w1a_d[:, :, f0:f0 + 512].rearrange(
                                "k p f -> p k f"
                            ),
                        )
                else:
                    nc.sync.dma_start(
                        out=w1_sb.rearrange("p k f -> p (k f)"),
                        in_=w1r_d[e - 1],
                    )
                w2_sb = wp.tile([128, KF * D], bf16, tag="w2")
                nc.sync.dma_start(out=w2_sb[:], in_=w2_d[e])
                xt_sbs[e], w1_sbs[e], w2_sbs[e] = xt_sb, w1_sb, w2_sb

            # slot 0 inputs issued first so PE can start ASAP
            load_slot(0, first=True)
            assert caps[0] > 0

            # biases (small / off critical path; b2 broadcast on gpsimd queue)
            b1_sb = bp.tile([128, E_LOCAL, KF], fp32)
            nc.gpsimd.dma_start(out=b1_sb[:], in_=b1_d[:].rearrange("e p f -> p e f"))
            b2_sb = bp.tile([128, E_LOCAL, D], fp32)
            b2_ap = b2_d[:]
            b2_bc = bass.AP(
                tensor=b2_ap.tensor,
                offset=b2_ap.offset,
                ap=[[0, 128]] + [list(a) for a in b2_ap.ap],
            )
            nc.gpsimd.dma_start(out=b2_sb[:], in_=b2_bc)

            for e in range(E_LOCAL):
                cap = caps[e]
                if cap == 0:
                    continue
                if e + 1 < E_LOCAL and caps[e + 1] > 0:
                    load_slot(e + 1, first=False)
                w1_sb, w2_sb, xt_sb = w1_sbs[e], w2_sbs[e], xt_sbs[e]

                # layer-1 token chunks (PSUM free dim <= 512 fp32).
                # Balanced halves for cap > 512: a tiny second chunk would
                # pay a full LDWEIGHTS per matmul for a handful of columns.
                if cap <= 512:
                    chunks = [(0, cap)]
                else:
                    h = (cap + 1) // 2
                    chunks = [(0, h), (h, cap - h)]

                # layer 1: HT[f-tile, tok] = gelu(w1_tile.T @ XT + b1)
                ht_sb = hp.tile([128, KF, CAPMAX], bf16, tag="ht")
                for f in range(KF):
                    for (c0, cs) in chunks:
                        ph = psh.tile([128, 512], fp32, tag="ph")
                        for k in range(KD):
                            nc.tensor.matmul(
                                ph[:, :cs],
                                lhsT=w1_sb[:, k, f * 128:(f + 1) * 128],
                                rhs=xt_sb[:, k * cap + c0:k * cap + c0 + cs],
                                start=(k == 0),
                                stop=(k == KD - 1),
                            )
                        nc.scalar.activation(
                            out=ht_sb[:, f, c0:c0 + cs],
                            in_=ph[:, :cs],
                            func=mybir.ActivationFunctionType.Gelu,
                            bias=b1_sb[:, e, f:f + 1],
                            scale=1.0,
                        )

                # layer 2: Y[t-tile, :] = HT_tile.T @ w2 + b2
                NT = -(-cap // 128)
                for t in range(NT):
                    tt = min(128, cap - t * 128)
                    py = psy.tile([128, D], fp32, tag="py")
                    for k in range(KF):
                        nc.tensor.matmul(
                            py[:tt, :],
                            lhsT=ht_sb[:, k, t * 128:t * 128 + tt],
                            rhs=w2_sb[:, k * D:(k + 1) * D],
                            start=(k == 0),
                            stop=(k == KF - 1),
                        )
                    y_sb = yp.tile([128, D], fp32, tag="ysb")
                    nc.vector.tensor_add(y_sb[:tt, :], py[:tt, :], b2_sb[:tt, e, :])
                    nc.sync.dma_start(
                        out=y_d[yoff[e] + t * 128: yoff[e] + t * 128 + tt, :],
                        in_=y_sb[:tt, :],
                    )

    nc.compile()
    return nc


def _get_nc(caps):
    key = tuple(caps)
    if key not in _nc_cache:
        _nc_cache[key] = _build_nc(key)
    return _nc_cache[key]


def kernel(**inputs):
    x = np.asarray(inputs["inputs"], dtype=np.float32)
    disp = np.asarray(inputs["dispatch_order"])
    w1 = np.asarray(inputs["w1"], dtype=np.float32)
    b1 = np.asarray(inputs["b1"], dtype=np.float32)
    w2 = np.asarray(inputs["w2"], dtype=np.float32)
    b2 = np.asarray(inputs["b2"], dtype=np.float32)

    B, S, Dd = x.shape
    assert Dd == D
    T = B * S
    xf = x.reshape(T, D)
    e = disp.astype(np.int64)

    counts = np.bincount(e, minlength=NUM_EXPERTS)
    order = np.argsort(e, kind="stable")
    xs = xf[order]  # tokens grouped by expert, original order within expert
    offs = np.zeros(NUM_EXPERTS + 1, dtype=np.int64)
    np.cumsum(counts, out=offs[1:])

    # assign experts to (slot, core): slot j of core c gets the (8j+c)-th
    # most-loaded expert -> tight per-slot caps, balanced cores
    by_load = np.argsort(-counts, kind="stable")
    slot_expert = by_load.reshape(E_LOCAL, N_CORES)  # [slot, core] -> expert id
    caps = tuple(int(counts[slot_expert[j]].max()) for j in range(E_LOCAL))
    xoff, yoff = _slot_geometry(caps)

    # weights in device layout (partition-major except slot-0 w1, which
    # stays k-major so the kernel can stream it in f-blocks at startup)
    w1b = w1.astype(_BF16).reshape(NUM_EXPERTS, KD, 128, F)
    w1p = np.ascontiguousarray(
        w1b.transpose(0, 2, 1, 3).reshape(NUM_EXPERTS, 128, KD * F)
    )
    w2p = np.ascontiguousarray(
        w2.astype(_BF16).reshape(NUM_EXPERTS, KF, 128, D)
        .transpose(0, 2, 1, 3).reshape(NUM_EXPERTS, 128, KF * D)
    )
    b1r = np.ascontiguousarray(
        b1.reshape(NUM_EXPERTS, KF, 128).transpose(0, 2, 1)
    )  # [E, 128, KF]
    xsb = xs.astype(_BF16)

    in_maps = []
    for c in range(N_CORES):
        eids = [int(slot_expert[j, c]) for j in range(E_LOCAL)]
        xt = np.zeros((128, KD * xoff[-1]), dtype=_BF16)
        for j, ei in enumerate(eids):
            cnt = int(counts[ei])
            cap = caps[j]
            if cnt:
                xe = xsb[offs[ei]:offs[ei + 1]]  # [cnt, D]
                xtj = xe.T.reshape(KD, 128, cnt).transpose(1, 0, 2)  # [128,KD,cnt]
                base = KD * xoff[j]
                for k in range(KD):
                    xt[:, base + k * cap:base + k * cap + cnt] = xtj[:, k, :]
        in_maps.append({
            "xt": xt,
            "w1a": np.ascontiguousarray(w1b[eids[0]]),
            "w1r": np.ascontiguousarray(w1p[eids[1:]]),
            "w2": np.ascontiguousarray(w2p[eids]),
            "b1": np.ascontiguousarray(b1r[eids]),
            "b2": np.ascontiguousarray(b2[eids]),
        })

    nc = _get_nc(caps)
    global _last_in_maps
    _last_in_maps = in_maps
    from concourse.bass_utils import run_bass_kernel_spmd

    res = run_bass_kernel_spmd(nc, in_maps, core_ids=list(range(N_CORES)))

    out_sorted = np.empty((T, D), dtype=np.float32)
    for c in range(N_CORES):
        y = res.results[c]["y"]
        for j in range(E_LOCAL):
            ei = int(slot_expert[j, c])
            cnt = int(counts[ei])
            if cnt:
                out_sorted[offs[ei]:offs[ei + 1]] = y[yoff[j]:yoff[j] + cnt]

    out = np.empty((T, D), dtype=np.float32)
    out[order] = out_sorted
    return out.reshape(B, S, D)


# revision 20
# speedup vs baseline: 1.0547x; 1.0547x over previous
"""MoE expert-parallel kernel for Trainium2 (8 NeuronCores).

Strategy:
  - Host: route tokens to experts (stable sort by dispatch_order). Experts are
    assigned to (core, slot) pairs by descending token count: slot j of core c
    gets the (8*j + c)-th most-loaded expert, so all cores see nearly identical
    work and slot j's capacity cap_j = max over cores of its count (tight).
  - Device (SPMD, 8 cores, 8 expert slots/core):
    per slot: HT = gelu(w1^T-tiled @ XT + b1) computed transposed [F, tokens],
    then Y = HT^T @ w2 + b2 [tokens, D]; bf16 operands, fp32 PSUM accumulation.
  - Host: scatter per-expert outputs back to original token order.

No cross-core collectives: each core owns a disjoint set of experts, hence a
disjoint set of output token rows.
"""

import sys

import numpy as np
import ml_dtypes

for _p in ("/opt/trn_rl_repo",):
    if _p not in sys.path:
        sys.path.insert(0, _p)

_BF16 = ml_dtypes.bfloat16

NUM_EXPERTS = 64
N_CORES = 8
E_LOCAL = NUM_EXPERTS // N_CORES  # 8 expert slots per core
D = 512
F = 2048
KD = D // 128   # 4 contraction tiles for layer 1
KF = F // 128   # 16 contraction tiles for layer 2

_nc_cache = {}


def _slot_geometry(caps):
    """Per-slot column offsets for xt and row offsets for y."""
    xoff = [0]
    yoff = [0]
    for c in caps:
        xoff.append(xoff[-1] + c)
        yoff.append(yoff[-1] + (-(-c // 128)) * 128)
    return xoff, yoff


def _build_nc(caps):
    """Build + compile the SPMD Bass program for per-slot capacities `caps`."""
    import concourse.bacc as bacc
    import concourse.bass as bass
    import concourse.mybir as mybir
    import concourse.tile as tile

    fp32 = mybir.dt.float32
    bf16 = mybir.dt.bfloat16

    xoff, yoff = _slot_geometry(caps)
    XCOLS = xoff[-1]
    YROWS = yoff[-1]
    CAPMAX = max(caps)

    nc = bacc.Bacc("TRN2", target_bir_lowering=False, debug=False)

    # xt/w1r/w2 are partition-major: one contiguous run per partition per
    # transfer -> 128 large DMA descriptors instead of 512-2048 small ones.
    xt_d = nc.dram_tensor("xt", [128, KD * XCOLS], bf16, kind="ExternalInput")
    w1a_d = nc.dram_tensor("w1a", [KD, 128, F], bf16, kind="ExternalInput")
    w1r_d = nc.dram_tensor(
        "w1r", [E_LOCAL - 1, 128, KD * F], bf16, kind="ExternalInput"
    )
    w2_d = nc.dram_tensor("w2", [E_LOCAL, 128, KF * D], bf16, kind="ExternalInput")
    b1_d = nc.dram_tensor("b1", [E_LOCAL, 128, KF], fp32, kind="ExternalInput")
    b2_d = nc.dram_tensor("b2", [E_LOCAL, D], fp32, kind="ExternalInput")
    y_d = nc.dram_tensor("y", [YROWS, D], fp32, kind="ExternalOutput")

    with tile.TileContext(nc) as tc:
        with (
            tc.tile_pool(name="wpool", bufs=2) as wp,
            tc.tile_pool(name="w2pool", bufs=4) as w2p,
            tc.tile_pool(name="rpool", bufs=1) as rp,
            tc.tile_pool(name="xpool", bufs=2) as xp,
            tc.tile_pool(name="hpool", bufs=2) as hp,
            tc.tile_pool(name="ypool", bufs=4) as yp,
            tc.tile_pool(name="bias", bufs=1) as bp,
            tc.tile_pool(name="psh", bufs=4, space="PSUM") as psh,
            tc.tile_pool(name="psy", bufs=3, space="PSUM") as psy,
        ):
            w1_sbs = [None] * E_LOCAL
            w2_sbs = [None] * E_LOCAL
            xt_sbs = [None] * E_LOCAL

            def load_slot(e, first):
                # Single HWDGE ring (SP): FIFO start order + packet-level
                # round-robin. Critical startup transfers (xt0, w1_0 chunks)
                # are issued first; everything else queues behind them.
                cap = caps[e]
                xt_sb = xp.tile([128, KD * cap], bf16, tag="xt")
                nc.sync.dma_start(
                    out=xt_sb[:],
                    in_=xt_d[:, KD * xoff[e]:KD * xoff[e + 1]],
                )
                w1_sb = wp.tile([128, KD, F], bf16, tag="w1")
                if first:
                    # progressive f-blocks so PE starts after ~1 MB
                    for f0 in range(0, F, 512):
                        nc.sync.dma_start(
                            out=w1_sb[:, :, f0:f0 + 512],
                            in_=w1a_d[:, :, f0:f0 + 512].rearrange(
                                "k p f -> p k f"
                            ),
                        )
                else:
                    nc.sync.dma_start(
                        out=w1_sb.rearrange("p k f -> p (k f)"),
                        in_=w1r_d[e - 1],
                    )
                w2_sb = w2p.tile([128, KF * D], bf16, tag="w2")
                nc.sync.dma_start(out=w2_sb[:], in_=w2_d[e])
                xt_sbs[e], w1_sbs[e], w2_sbs[e] = xt_sb, w1_sb, w2_sb

            # slot 0 inputs issued first so PE can start ASAP
            load_slot(0, first=True)
            assert caps[0] > 0

            # biases (small / off critical path; b2 broadcast on gpsimd queue)
            b1_sb = bp.tile([128, E_LOCAL, KF], fp32)
            nc.gpsimd.dma_start(out=b1_sb[:], in_=b1_d[:].rearrange("e p f -> p e f"))
            b2_sb = bp.tile([128, E_LOCAL, D], fp32)
            b2_ap = b2_d[:]
            b2_bc = bass.AP(
                tensor=b2_ap.tensor,
                offset=b2_ap.offset,
                ap=[[0, 128]] + [list(a) for a in b2_ap.ap],
            )
            nc.gpsimd.dma_start(out=b2_sb[:], in_=b2_bc)

            # Layer-2 partial tiles cost a full 16x512-cycle pass no matter
            # how few tokens they hold. Pack the remainder tokens of 3-slot
            # windows into <=32-token column groups and run up to 4 groups
            # concurrently in one PE pass (column tiling, tile_position
            # derived automatically from the PSUM base partition).
            WINDOWS = [(0, 3), (3, 6)]  # slots 6-7 keep their partial tiles
            packed = set()
            win_pieces = {}
            for w0, w1e in WINDOWS:
                pieces = []  # (slot, ht_col0, m, rbase)
                rbase = sum(
                    -(-(caps[s] % 128) // 32) * 32
                    for ww0, ww1 in WINDOWS if (ww0, ww1) < (w0, w1e)
                    for s in range(ww0, ww1) if caps[s] % 128
                )
                for s in range(w0, min(w1e, E_LOCAL)):
                    rem = caps[s] % 128
                    if rem == 0 or caps[s] == 0:
                        continue
                    full = caps[s] // 128
                    off = 0
                    while off < rem:
                        m = min(32, rem - off)
                        pieces.append((s, full * 128 + off, m, rbase + off))
                        off += m
                    rbase += -(-rem // 32) * 32
                n_passes = -(-len(pieces) // 4)
                n_slots = len({p[0] for p in pieces})
                if pieces and n_passes < n_slots:
                    win_pieces[(w0, w1e)] = pieces
                    packed.update({p[0] for p in pieces})
            RTOT = sum(
                -(-(caps[s] % 128) // 32) * 32
                for w0, w1e in win_pieces
                for s in range(w0, min(w1e, E_LOCAL)) if caps[s] % 128
            )
            r_sb = rp.tile([128, KF, max(RTOT, 32)], bf16, name="r_sb") if win_pieces else None
            ht_sbs = {}

            def packed_pass(w0, w1e):
                pieces = win_pieces[(w0, w1e)]
                for i0 in range(0, len(pieces), 4):
                    grp = pieces[i0:i0 + 4]
                    py = psy.tile([128, D], fp32, tag="py")
                    for k in range(KF):
                        for gi, (s, hc0, m, rb) in enumerate(grp):
                            nc.tensor.matmul(
                                py[32 * gi:32 * gi + m, :],
                                lhsT=r_sb[:, k, rb:rb + m],
                                rhs=w2_sbs[s][:, k * D:(k + 1) * D],
                                start=(k == 0),
                                stop=(k == KF - 1),
                                tile_position=(0, 32 * gi),
                            )
                    y_sb = yp.tile([128, D], fp32, tag="ysb")
                    for gi, (s, hc0, m, rb) in enumerate(grp):
                        nc.vector.tensor_add(
                            y_sb[32 * gi:32 * gi + m, :],
                            py[32 * gi:32 * gi + m, :],
                            b2_sb[32 * gi:32 * gi + m, s, :],
                        )
                        row0 = yoff[s] + (caps[s] // 128) * 128 + (hc0 - (caps[s] // 128) * 128)
                        nc.sync.dma_start(
                            out=y_d[row0:row0 + m, :],
                            in_=y_sb[32 * gi:32 * gi + m, :],
                        )

            for e in range(E_LOCAL):
                cap = caps[e]
                if cap == 0:
                    continue
                if e + 1 < E_LOCAL and caps[e + 1] > 0:
                    load_slot(e + 1, first=False)
                w1_sb, w2_sb, xt_sb = w1_sbs[e], w2_sbs[e], xt_sbs[e]

                # layer-1 token chunks (PSUM free dim <= 512 fp32).
                # Balanced halves for cap > 512: a tiny second chunk would
                # pay a full LDWEIGHTS per matmul for a handful of columns.
                if cap <= 512:
                    chunks = [(0, cap)]
                else:
                    h = (cap + 1) // 2
                    chunks = [(0, h), (h, cap - h)]

                # layer 1: HT[f-tile, tok] = gelu(w1_tile.T @ XT + b1)
                ht_sb = hp.tile([128, KF, CAPMAX], bf16, tag="ht")
                for f in range(KF):
                    for (c0, cs) in chunks:
                        ph = psh.tile([128, 512], fp32, tag="ph")
                        for k in range(KD):
                            nc.tensor.matmul(
                                ph[:, :cs],
                                lhsT=w1_sb[:, k, f * 128:(f + 1) * 128],
                                rhs=xt_sb[:, k * cap + c0:k * cap + c0 + cs],
                                start=(k == 0),
                                stop=(k == KD - 1),
                            )
                        nc.scalar.activation(
                            out=ht_sb[:, f, c0:c0 + cs],
                            in_=ph[:, :cs],
                            func=mybir.ActivationFunctionType.Gelu,
                            bias=b1_sb[:, e, f:f + 1],
                            scale=1.0,
                        )

                if e in packed:
                    rem = cap % 128
                    rb0 = None
                    for (s, hc0, m, rb) in [p for w in win_pieces.values() for p in w]:
                        if s == e:
                            rb0 = rb
                            break
                    nc.vector.tensor_copy(
                        r_sb[:, :, rb0:rb0 + rem],
                        ht_sb[:, :, (cap // 128) * 128:cap],
                    )

                # layer 2: Y[t-tile, :] = HT_tile.T @ w2 + b2
                NT = cap // 128 if e in packed else -(-cap // 128)
                for t in range(NT):
                    tt = min(128, cap - t * 128)
                    py = psy.tile([128, D], fp32, tag="py")
                    for k in range(KF):
                        nc.tensor.matmul(
                            py[:tt, :],
                            lhsT=ht_sb[:, k, t * 128:t * 128 + tt],
                            rhs=w2_sb[:, k * D:(k + 1) * D],
                            start=(k == 0),
                            stop=(k == KF - 1),
                        )
                    y_sb = yp.tile([128, D], fp32, tag="ysb")
                    nc.vector.tensor_add(y_sb[:tt, :], py[:tt, :], b2_sb[:tt, e, :])
                    nc.sync.dma_start(
                        out=y_d[yoff[e] + t * 128: yoff[e] + t * 128 + tt, :],
                        in_=y_sb[:tt, :],
                    )

                for (w0, w1e) in list(win_pieces):
                    if e == min(w1e, E_LOCAL) - 1:
                        packed_pass(w0, w1e)

    nc.compile()
    return nc


def _get_nc(caps):
    key = tuple(caps)
    if key not in _nc_cache:
        _nc_cache[key] = _build_nc(key)
    return _nc_cache[key]


def kernel(**inputs):
    x = np.asarray(inputs["inputs"], dtype=np.float32)
    disp = np.asarray(inputs["dispatch_order"])
    w1 = np.asarray(inputs["w1"], dtype=np.float32)
    b1 = np.asarray(inputs["b1"], dtype=np.float32)
    w2 = np.asarray(inputs["w2"], dtype=np.float32)
    b2 = np.asarray(inputs["b2"], dtype=np.float32)

    B, S, Dd = x.shape
    assert Dd == D
    T = B * S
    xf = x.reshape(T, D)
    e = disp.astype(np.int64)

    counts = np.bincount(e, minlength=NUM_EXPERTS)
    order = np.argsort(e, kind="stable")
    xs = xf[order]  # tokens grouped by expert, original order within expert
    offs = np.zeros(NUM_EXPERTS + 1, dtype=np.int64)
    np.cumsum(counts, out=offs[1:])

    # assign experts to (slot, core): slot j of core c gets the (8j+c)-th
    # most-loaded expert -> tight per-slot caps, balanced cores
    by_load = np.argsort(-counts, kind="stable")
    slot_expert = by_load.reshape(E_LOCAL, N_CORES)  # [slot, core] -> expert id
    caps = tuple(int(counts[slot_expert[j]].max()) for j in range(E_LOCAL))
    xoff, yoff = _slot_geometry(caps)

    # weights in device layout (partition-major except slot-0 w1, which
    # stays k-major so the kernel can stream it in f-blocks at startup)
    w1b = w1.astype(_BF16).reshape(NUM_EXPERTS, KD, 128, F)
    w1p = np.ascontiguousarray(
        w1b.transpose(0, 2, 1, 3).reshape(NUM_EXPERTS, 128, KD * F)
    )
    w2p = np.ascontiguousarray(
        w2.astype(_BF16).reshape(NUM_EXPERTS, KF, 128, D)
        .transpose(0, 2, 1, 3).reshape(NUM_EXPERTS, 128, KF * D)
    )
    b1r = np.ascontiguousarray(
        b1.reshape(NUM_EXPERTS, KF, 128).transpose(0, 2, 1)
    )  # [E, 128, KF]
    xsb = xs.astype(_BF16)

    in_maps = []
    for c in range(N_CORES):
        eids = [int(slot_expert[j, c]) for j in range(E_LOCAL)]
        xt = np.zeros((128, KD * xoff[-1]), dtype=_BF16)
        for j, ei in enumerate(eids):
            cnt = int(counts[ei])
            cap = caps[j]
            if cnt:
                xe = xsb[offs[ei]:offs[ei + 1]]  # [cnt, D]
                xtj = xe.T.reshape(KD, 128, cnt).transpose(1, 0, 2)  # [128,KD,cnt]
                base = KD * xoff[j]
                for k in range(KD):
                    xt[:, base + k * cap:base + k * cap + cnt] = xtj[:, k, :]
        in_maps.append({
            "xt": xt,
            "w1a": np.ascontiguousarray(w1b[eids[0]]),
            "w1r": np.ascontiguousarray(w1p[eids[1:]]),
            "w2": np.ascontiguousarray(w2p[eids]),
            "b1": np.ascontiguousarray(b1r[eids]),
            "b2": np.ascontiguousarray(b2[eids]),
        })

    nc = _get_nc(caps)
    global _last_in_maps
    _last_in_maps = in_maps
    from concourse.bass_utils import run_bass_kernel_spmd

    res = run_bass_kernel_spmd(nc, in_maps, core_ids=list(range(N_CORES)))

    out_sorted = np.empty((T, D), dtype=np.float32)
    for c in range(N_CORES):
        y = res.results[c]["y"]
        for j in range(E_LOCAL):
            ei = int(slot_expert[j, c])
            cnt = int(counts[ei])
            if cnt:
                out_sorted[offs[ei]:offs[ei + 1]] = y[yoff[j]:yoff[j] + cnt]

    out = np.empty((T, D), dtype=np.float32)
    out[order] = out_sorted
    return out.reshape(B, S, D)
